# revision 8
# baseline (speedup 1.0000x reference)
"""Trainium2 Bass kernel for nn_Coefficients: assemble the MNA coefficient
block matrix  [[M, 0, 0], [0, I, -M^T], [diag(z), diag(y), 0]]  of shape
[N+2E, 2E+N] from M [N,E], params/kinds/sw_params.

Sharding (8 cores, SPMD — one program, per-core data):
  core c owns kcl rows [128c,128c+128), kvl rows e in [256c,256c+256) and
  elem rows e in the same range.  Each core writes a local out_main
  [640, 5120] (its kcl/kvl/elem row chunks, with zeros where the
  core-dependent diagonal bands go) plus out_bands [768, 256]: the three
  256x256 diagonal blocks (identity, diag(z), diag(y)) whose global column
  position depends on the core; the host unshard step places rows and
  overlays bands into the full [5120, 5120] output.

The toolchain allows only one sync-wait per instruction, so the kernel is
structured as <=8 HWDGE DMAs (no DMA sem-lane reuse) each with at most one
producer dependency.
"""

import numpy as np

N, E, SIG = 1024, 2048, 64
C = 8            # cores
RK = N // C      # 128 kcl rows per core
RE = E // C      # 256 kvl/elem rows per core
W = 2 * E + N    # 5120 output width
DT = 1e-6

_cache = {}


def _build_nc():
    import concourse.bass as bass
    import concourse.mybir as mybir
    from concourse.tile import TileContext

    f32 = mybir.dt.float32
    nc = bass.Bass(name="coeffs_scatter")

    mrow = nc.dram_tensor("mrow", [RK, E], f32, kind="ExternalInput")
    negmt = nc.dram_tensor("negmt", [RE, N], f32, kind="ExternalInput")
    # Pre-broadcast diagonal values [128, 512]: cols 0:256 z, 256:512 y;
    # within each 256, col block k in {0,1} holds vals[128k + p] at row p.
    vb = nc.dram_tensor("vb", [128, 2 * RE], f32, kind="ExternalInput")

    out_main = nc.dram_tensor("out_main", [RK + 2 * RE, W], f32, kind="ExternalOutput")
    # rows 0:256 identity band, 256:512 diag(z) band, 512:768 diag(y) band
    out_bands = nc.dram_tensor("out_bands", [3 * RE, RE], f32, kind="ExternalOutput")

    ZW = 1024  # zero-tile width; every zero span is a multiple of this

    with TileContext(nc) as tc:
        with tc.tile_pool(name="pool", bufs=1) as pool:
            # Band value load first on the SP ring (small; its consumers are
            # the affine_selects feeding the band DMA).
            vbt = pool.tile([128, 2 * RE], f32, tag="vbt")
            nc.sync.dma_start(out=vbt[:], in_=vb[:, :])

            # Big DRAM->DRAM copies: M rows into the kcl block, -M^T rows
            # into the kvl right block.  No deps, start immediately.
            nc.sync.dma_start(out=out_main[0:RK, 0:E], in_=mrow[:, :])
            nc.scalar.dma_start(out=out_main[RK:RK + RE, 2 * E:W], in_=negmt[:, :])

            ones = pool.tile([128, RE], f32, tag="ones")
            nc.vector.memset(ones[:], 1.0)

            # Zero source tile, read repeatedly (broadcast AP) by the
            # zero-fill DMAs.  Kept small so the memset gate is short.
            zt = pool.tile([128, ZW], f32, tag="zt")
            nc.vector.memset(zt[:], 0.0)

            # Zero fills: one [128, width] DMA per 128-row chunk, zt
            # broadcast along the width.  Alternate rings for balance:
            # SP gets elem0, kvl1; ACT gets kcl, kvl0, elem1.
            def zfill(engine, row0, col0, width):
                b = width // ZW
                dst = out_main[row0:row0 + 128, col0:col0 + width] \
                    .rearrange("p (b c) -> p b c", c=ZW)
                src = zt[:, :].rearrange("p (b c) -> p b c", b=1) \
                    .broadcast_to([128, b, ZW])
                engine.dma_start(out=dst, in_=src)

            zfill(nc.sync, RK + RE, 0, W)           # elem rows 0:128
            zfill(nc.scalar, 0, E, W - E)           # kcl rows, cols E:W
            zfill(nc.scalar, RK, 0, 2 * E)          # kvl rows 0:128, cols 0:2E
            zfill(nc.sync, RK + 128, 0, 2 * E)      # kvl rows 128:256
            zfill(nc.scalar, RK + RE + 128, 0, W)   # elem rows 128:256

            # Six [128, 256] half-bands via affine_select: keep in_[p, c]
            # where c - p - 128k == 0, fill 0.  Result is [diag|0] (k=0) or
            # [0|diag] (k=1).
            bt = pool.tile([128, 6 * RE], f32, tag="bt")
            srcs = [ones[:, :], ones[:, :],
                    vbt[:, 0:RE], vbt[:, 0:RE],
                    vbt[:, RE:2 * RE], vbt[:, RE:2 * RE]]
            for j, src in enumerate(srcs):
                nc.gpsimd.affine_select(
                    bt[:, RE * j:RE * (j + 1)], src,
                    pattern=[[1, RE]],
                    compare_op=mybir.AluOpType.is_equal,
                    fill=0.0, base=-128 * (j % 2), channel_multiplier=-1,
                )

            # One DMA for all three bands: [128, 6, 256] -> [6, 128, 256].
            # Last on its ring so its producer wait can't delay other issues.
            nc.sync.dma_start(
                out=out_bands.rearrange("(j p) c -> p j c", p=128),
                in_=bt[:, :].rearrange("p (j c) -> p j c", j=6),
            )

    _split_waits(nc)
    return nc


def _split_waits(nc, maxw=1):
    """This walrus build rejects instructions carrying more than one
    sync-wait ("Too many sync wait commands").  Tile can emit several on one
    instruction (notably the kernel-tail Drain).  Hoist the extras onto
    same-engine NoOps inserted immediately before the instruction."""
    import concourse.mybir as mybir

    nsplit = 0
    for fn in nc.m.functions:
        for blk in fn.blocks:
            newlist = []
            changed = False
            for inst in blk.instructions:
                si = inst.sync_info
                ow = list(si.on_wait) if si is not None and si.on_wait else []
                if len(ow) > maxw:
                    head, tail = ow[:-maxw], ow[-maxw:]
                    for w in head:
                        nop = mybir.InstNoOp(name=f"nopw-{nsplit}", ins=[], outs=[])
                        nsplit += 1
                        nop.engine = inst.engine
                        nop.sync_info = mybir.SyncInfo(on_wait=[w], on_update=[])
                        newlist.append(nop)
                    inst.sync_info = mybir.SyncInfo(
                        on_wait=tail,
                        on_update=list(si.on_update) if si.on_update else [])
                    changed = True
                newlist.append(inst)
            if changed:
                blk.instructions = newlist
    return nsplit


def _element_vals(params, sw_params, kinds, time):
    """Host replica of reference._element_vals (numpy, f32)."""
    params = np.asarray(params, dtype=np.float32)
    sw_params = np.asarray(sw_params, dtype=np.float32)
    kinds = np.asarray(kinds)
    t = int(time)
    sw_on = sw_params[:, t] > 0  # sigmoid(x) > 0.5  <=>  x > 0
    one = np.ones_like(params)
    zero = np.zeros_like(params)
    ndt = (np.float32(-DT) / params).astype(np.float32)
    z_vals = np.select(
        [kinds == 0, kinds == 1, kinds == 2, kinds == 3, kinds == 4, kinds == 5],
        [-params, zero, one, np.where(sw_on, 0.0, 1.0).astype(np.float32), ndt, one],
    ).astype(np.float32)
    y_vals = np.select(
        [kinds == 0, kinds == 1, kinds == 2, kinds == 3, kinds == 4, kinds == 5],
        [one, one, zero, np.where(sw_on, 1.0, 0.0).astype(np.float32), one, ndt],
    ).astype(np.float32)
    return z_vals, y_vals


def _run(M, params, sw_params, kinds, time, trace=False):
    from concourse.bass_utils import run_bass_kernel_spmd

    M = np.ascontiguousarray(np.asarray(M, dtype=np.float32))
    z_vals, y_vals = _element_vals(params, sw_params, kinds, time)
    negMt = -(M.T)  # [E, N] C-contiguous

    in_maps = []
    for c in range(C):
        # [128, 256] broadcast: col block k (=c//128) holds vals[128k + p].
        zc = np.repeat(z_vals[RE * c:RE * (c + 1)].reshape(2, 128).T, 128, axis=1)
        yc = np.repeat(y_vals[RE * c:RE * (c + 1)].reshape(2, 128).T, 128, axis=1)
        in_maps.append({
            "mrow": M[RK * c:RK * (c + 1), :],
            "negmt": negMt[RE * c:RE * (c + 1), :],
            "vb": np.ascontiguousarray(np.concatenate([zc, yc], axis=1)),
        })

    if "nc" not in _cache:
        _cache["nc"] = _build_nc()
    res = run_bass_kernel_spmd(
        _cache["nc"], in_maps, core_ids=list(range(C)), trace=trace,
        trace_cores=list(range(C)) if trace else None,
    )

    full = np.empty((N + 2 * E, 2 * E + N), dtype=np.float32)
    for c in range(C):
        r = res.results[c]
        om = r["out_main"]
        full[RK * c:RK * (c + 1), :] = om[0:RK]
        full[N + RE * c:N + RE * (c + 1), :] = om[RK:RK + RE]
        full[N + E + RE * c:N + E + RE * (c + 1), :] = om[RK + RE:RK + 2 * RE]
        # overlay core-dependent diagonal bands
        bands = r["out_bands"]
        full[N + RE * c:N + RE * (c + 1), E + RE * c:E + RE * (c + 1)] = bands[0:RE]
        full[N + E + RE * c:N + E + RE * (c + 1), RE * c:RE * (c + 1)] = bands[RE:2 * RE]
        full[N + E + RE * c:N + E + RE * (c + 1), E + RE * c:E + RE * (c + 1)] = bands[2 * RE:3 * RE]
    return full, res


def kernel(M, params, sw_params, kinds, time):
    out, _ = _run(M, params, sw_params, kinds, time, trace=False)
    return out


# revision 10
# speedup vs baseline: 1.0497x; 1.0497x over previous
"""Trainium2 Bass kernel for nn_Coefficients: assemble the MNA coefficient
block matrix  [[M, 0, 0], [0, I, -M^T], [diag(z), diag(y), 0]]  of shape
[N+2E, 2E+N] from M [N,E], params/kinds/sw_params.

Sharding (8 cores, SPMD — one program, per-core data):
  core c owns kcl rows [128c,128c+128), kvl rows e in [256c,256c+256) and
  elem rows e in the same range.  Each core writes a local out_main
  [640, 5120] (its kcl/kvl/elem row chunks, with zeros where the
  core-dependent diagonal bands go) plus out_bands [768, 256]: the three
  256x256 diagonal blocks (identity, diag(z), diag(y)) whose global column
  position depends on the core; the host unshard step places rows and
  overlays bands into the full [5120, 5120] output.

The toolchain allows only one sync-wait per instruction, so the kernel is
structured as <=8 HWDGE DMAs (no DMA sem-lane reuse) each with at most one
producer dependency.
"""

import numpy as np

N, E, SIG = 1024, 2048, 64
C = 8            # cores
RK = N // C      # 128 kcl rows per core
RE = E // C      # 256 kvl/elem rows per core
W = 2 * E + N    # 5120 output width
DT = 1e-6

_cache = {}


def _build_nc():
    import concourse.bass as bass
    import concourse.mybir as mybir
    from concourse.tile import TileContext

    f32 = mybir.dt.float32
    nc = bass.Bass(name="coeffs_scatter")

    mrow = nc.dram_tensor("mrow", [RK, E], f32, kind="ExternalInput")
    negmt = nc.dram_tensor("negmt", [RE, N], f32, kind="ExternalInput")
    # Pre-broadcast diagonal values [128, 512]: cols 0:256 z, 256:512 y;
    # within each 256, col block k in {0,1} holds vals[128k + p] at row p.
    vb = nc.dram_tensor("vb", [128, 2 * RE], f32, kind="ExternalInput")

    out_main = nc.dram_tensor("out_main", [RK + 2 * RE, W], f32, kind="ExternalOutput")
    # rows 0:256 identity band, 256:512 diag(z) band, 512:768 diag(y) band
    out_bands = nc.dram_tensor("out_bands", [3 * RE, RE], f32, kind="ExternalOutput")

    with TileContext(nc) as tc:
        with tc.tile_pool(name="pool", bufs=1) as pool:
            # Band value load first on the SP ring (small; its consumers are
            # the affine_selects feeding the band DMA).
            vbt = pool.tile([128, 2 * RE], f32, tag="vbt")
            nc.sync.dma_start(out=vbt[:], in_=vb[:, :])

            # Big DRAM->DRAM copies: M rows into the kcl block, -M^T rows
            # into the kvl right block.  No deps, start immediately.
            nc.sync.dma_start(out=out_main[0:RK, 0:E], in_=mrow[:, :])
            nc.scalar.dma_start(out=out_main[RK:RK + RE, 2 * E:W], in_=negmt[:, :])

            ones = pool.tile([128, RE], f32, tag="ones")
            nc.vector.memset(ones[:], 1.0)

            # Zero source tile, read repeatedly (broadcast AP) by the
            # zero-fill DMAs.  Full output width so zero-fill descriptors
            # stay large (20 KB); the memset overlaps the M/M^T copies.
            zt = pool.tile([128, W], f32, tag="zt")
            nc.vector.memset(zt[:], 0.0)

            # Zero fills: one DMA per block region; 256-row regions use a
            # 3D AP with the 128-row chunk index broadcast on the zt side.
            def zfill(engine, row0, nrows, col0, width):
                k = nrows // 128
                dst = out_main[row0:row0 + nrows, col0:col0 + width] \
                    .rearrange("(k p) c -> p k c", p=128)
                src = zt[:, 0:width].rearrange("p (k c) -> p k c", k=1) \
                    .broadcast_to([128, k, width])
                engine.dma_start(out=dst, in_=src)

            zfill(nc.sync, RK + RE, RE, 0, W)         # elem rows, full width
            zfill(nc.scalar, 0, RK, E, W - E)         # kcl rows, cols E:W
            zfill(nc.scalar, RK, RE, 0, 2 * E)        # kvl rows, cols 0:2E

            # Six [128, 256] half-bands via affine_select: keep in_[p, c]
            # where c - p - 128k == 0, fill 0.  Result is [diag|0] (k=0) or
            # [0|diag] (k=1).
            bt = pool.tile([128, 6 * RE], f32, tag="bt")
            srcs = [ones[:, :], ones[:, :],
                    vbt[:, 0:RE], vbt[:, 0:RE],
                    vbt[:, RE:2 * RE], vbt[:, RE:2 * RE]]
            for j, src in enumerate(srcs):
                nc.gpsimd.affine_select(
                    bt[:, RE * j:RE * (j + 1)], src,
                    pattern=[[1, RE]],
                    compare_op=mybir.AluOpType.is_equal,
                    fill=0.0, base=-128 * (j % 2), channel_multiplier=-1,
                )

            # One DMA for all three bands: [128, 6, 256] -> [6, 128, 256].
            # Last on its ring so its producer wait can't delay other issues.
            nc.sync.dma_start(
                out=out_bands.rearrange("(j p) c -> p j c", p=128),
                in_=bt[:, :].rearrange("p (j c) -> p j c", j=6),
            )

    _split_waits(nc)
    return nc


def _split_waits(nc, maxw=1):
    """This walrus build rejects instructions carrying more than one
    sync-wait ("Too many sync wait commands").  Tile can emit several on one
    instruction (notably the kernel-tail Drain).  Hoist the extras onto
    same-engine NoOps inserted immediately before the instruction."""
    import concourse.mybir as mybir

    nsplit = 0
    for fn in nc.m.functions:
        for blk in fn.blocks:
            newlist = []
            changed = False
            for inst in blk.instructions:
                si = inst.sync_info
                ow = list(si.on_wait) if si is not None and si.on_wait else []
                if len(ow) > maxw:
                    head, tail = ow[:-maxw], ow[-maxw:]
                    for w in head:
                        nop = mybir.InstNoOp(name=f"nopw-{nsplit}", ins=[], outs=[])
                        nsplit += 1
                        nop.engine = inst.engine
                        nop.sync_info = mybir.SyncInfo(on_wait=[w], on_update=[])
                        newlist.append(nop)
                    inst.sync_info = mybir.SyncInfo(
                        on_wait=tail,
                        on_update=list(si.on_update) if si.on_update else [])
                    changed = True
                newlist.append(inst)
            if changed:
                blk.instructions = newlist
    return nsplit


def _element_vals(params, sw_params, kinds, time):
    """Host replica of reference._element_vals (numpy, f32)."""
    params = np.asarray(params, dtype=np.float32)
    sw_params = np.asarray(sw_params, dtype=np.float32)
    kinds = np.asarray(kinds)
    t = int(time)
    sw_on = sw_params[:, t] > 0  # sigmoid(x) > 0.5  <=>  x > 0
    one = np.ones_like(params)
    zero = np.zeros_like(params)
    ndt = (np.float32(-DT) / params).astype(np.float32)
    z_vals = np.select(
        [kinds == 0, kinds == 1, kinds == 2, kinds == 3, kinds == 4, kinds == 5],
        [-params, zero, one, np.where(sw_on, 0.0, 1.0).astype(np.float32), ndt, one],
    ).astype(np.float32)
    y_vals = np.select(
        [kinds == 0, kinds == 1, kinds == 2, kinds == 3, kinds == 4, kinds == 5],
        [one, one, zero, np.where(sw_on, 1.0, 0.0).astype(np.float32), one, ndt],
    ).astype(np.float32)
    return z_vals, y_vals


def _run(M, params, sw_params, kinds, time, trace=False):
    from concourse.bass_utils import run_bass_kernel_spmd

    M = np.ascontiguousarray(np.asarray(M, dtype=np.float32))
    z_vals, y_vals = _element_vals(params, sw_params, kinds, time)
    negMt = -(M.T)  # [E, N] C-contiguous

    in_maps = []
    for c in range(C):
        # [128, 256] broadcast: col block k (=c//128) holds vals[128k + p].
        zc = np.repeat(z_vals[RE * c:RE * (c + 1)].reshape(2, 128).T, 128, axis=1)
        yc = np.repeat(y_vals[RE * c:RE * (c + 1)].reshape(2, 128).T, 128, axis=1)
        in_maps.append({
            "mrow": M[RK * c:RK * (c + 1), :],
            "negmt": negMt[RE * c:RE * (c + 1), :],
            "vb": np.ascontiguousarray(np.concatenate([zc, yc], axis=1)),
        })

    if "nc" not in _cache:
        _cache["nc"] = _build_nc()
    res = run_bass_kernel_spmd(
        _cache["nc"], in_maps, core_ids=list(range(C)), trace=trace,
        trace_cores=list(range(C)) if trace else None,
    )

    full = np.empty((N + 2 * E, 2 * E + N), dtype=np.float32)
    for c in range(C):
        r = res.results[c]
        om = r["out_main"]
        full[RK * c:RK * (c + 1), :] = om[0:RK]
        full[N + RE * c:N + RE * (c + 1), :] = om[RK:RK + RE]
        full[N + E + RE * c:N + E + RE * (c + 1), :] = om[RK + RE:RK + 2 * RE]
        # overlay core-dependent diagonal bands
        bands = r["out_bands"]
        full[N + RE * c:N + RE * (c + 1), E + RE * c:E + RE * (c + 1)] = bands[0:RE]
        full[N + E + RE * c:N + E + RE * (c + 1), RE * c:RE * (c + 1)] = bands[RE:2 * RE]
        full[N + E + RE * c:N + E + RE * (c + 1), E + RE * c:E + RE * (c + 1)] = bands[2 * RE:3 * RE]
    return full, res


def kernel(M, params, sw_params, kinds, time):
    out, _ = _run(M, params, sw_params, kinds, time, trace=False)
    return out
